# revision 21
# baseline (speedup 1.0000x reference)
"""GQA attention block (qk-rmsnorm + RoPE + causal GQA attention + out-proj),
tensor-parallel over 8 NeuronCores: 2-way data parallel (batch) x 4-way head
parallel (8 q heads / 2 kv heads per core). All-reduce of out-proj partials is
done on host (sum of 4 partials per batch).

Schedule: one globally-ordered macro-op stream interleaves projection tiles,
attention steps (S -> exp -> PV per 128-key tile, both kv heads row-tiled on
the PE concurrently), per-chain softmax normalizes, and out-proj psum waves,
so the PE never idles long enough for HAM to re-throttle. ACT runs only
{Exp, Ln, Square} (one activation table, zero reloads); DVE handles all
PSUM-touching vector work; Pool (no PSUM port) gets SBUF-only rope/reduce.
"""
import sys
import numpy as np

sys.path.insert(0, "/opt/trn_rl_repo")

import concourse.bass as bass  # noqa: E402
import concourse.bacc as bacc  # noqa: E402
import concourse.mybir as mybir  # noqa: E402
import concourse.tile as tile  # noqa: E402
from concourse import masks  # noqa: E402
from concourse.bass_utils import run_bass_kernel_spmd  # noqa: E402

f32 = mybir.dt.float32
f32r = mybir.dt.float32r
f16 = mybir.dt.float16
FT = mybir.ActivationFunctionType
AX = mybir.AxisListType

P = 128
T = 2048
H = 2048
D = 64
NQ = 8          # q heads per core
DQ = NQ * D     # 512
NTT = T // P    # 16 T tiles
NHC = H // P    # 16 hidden chunks
NBLK = 4        # T_q blocks of 512
BLK = 512
EPS = 1e-5
MASKVAL = -30000.0
LN64 = -4.1588830833596715  # ln(1/64): scales exp to keep 1/rowsum in f16 normal range
L_PV = 4        # PV lag in attention steps
LN2 = 0.6931471805599453
RSQ_S0 = -0.5 * LN2 / (1 << 23)          # rsqrt seed: exp(s0*bits + b0)
RSQ_B0 = 0.5 * 127.0 * LN2 + 0.5 * 0.0430 * LN2

_CACHE = {}


def _build_program():
    nc = bacc.Bacc("TRN2", target_bir_lowering=False, debug=False, num_devices=8)

    xtt_d = nc.dram_tensor("xtt", [T, H], f16, kind="ExternalInput")
    wqkv_d = nc.dram_tensor("wqkv", [H, 768], f16, kind="ExternalInput")
    wo_d = nc.dram_tensor("wo", [DQ, H], f16, kind="ExternalInput")
    ropeq_d = nc.dram_tensor("ropeq", [P, 16 * 128], f16, kind="ExternalInput")
    ropek_d = nc.dram_tensor("ropek", [P, 16 * 128], f16, kind="ExternalInput")
    mtab_d = nc.dram_tensor("mtab", [P, 256], f16, kind="ExternalInput")
    out_d = nc.dram_tensor("out", [T, H], f16, kind="ExternalOutput")

    with tile.TileContext(nc) as tc:
        with (
            tc.tile_pool(name="persist", bufs=1) as pp,
            tc.tile_pool(name="work", bufs=2) as wp,
            tc.tile_pool(name="ptp", bufs=6) as ptp,
            tc.tile_pool(name="obp", bufs=12) as obp,
            tc.tile_pool(name="psum", bufs=2, space="PSUM") as ps,
            tc.tile_pool(name="psum_o", bufs=4, space="PSUM") as pop,
        ):
            # ---------- persistent tiles + input DMAs (arrival-ordered) ----------
            mtab = pp.tile([P, 256], f16, tag="mtab")
            nc.sync.dma_start(mtab[:], mtab_d[:])
            negI = mtab[:, 0:128]     # -30000 on diagonal
            ustr = mtab[:, 128:256]   # 1 where k > q (strict lower)

            wqkv_sb = [pp.tile([P, 768], f16, tag=f"wqkv{hc}", name=f"wqkv{hc}")
                       for hc in range(NHC)]
            xt_sb = [pp.tile([P, H], f16, tag=f"xt{tt}", name=f"xt{tt}")
                     for tt in range(NTT)]
            ropeq_sb = pp.tile([P, 16 * 128], f16, tag="ropeq")
            ropek_sb = pp.tile([P, 16 * 128], f16, tag="ropek")
            wo_sb = [pp.tile([P, H], f16, tag=f"woW{c}", name=f"woW{c}")
                     for c in range(4)]

            # weights + tables stream on the ACT HWDGE queue, x tiles on SP:
            # the two rings run in parallel and the latency-critical first
            # tile (wqkv0 + xtt0) lands in ~3us
            nc.sync.dma_start(xt_sb[0][:], xtt_d[0:P, :])
            for hc in range(NHC):
                nc.scalar.dma_start(wqkv_sb[hc][:], wqkv_d[hc * P:(hc + 1) * P, :])
            nc.scalar.dma_start(ropeq_sb[:], ropeq_d[:])
            nc.scalar.dma_start(ropek_sb[:], ropek_d[:])
            for tt in range(1, 8):
                nc.sync.dma_start(xt_sb[tt][:], xtt_d[tt * P:(tt + 1) * P, :])
            for c in range(4):
                nc.sync.dma_start(wo_sb[c][:], wo_d[c * P:(c + 1) * P, :])
            for tt in range(8, NTT):
                nc.sync.dma_start(xt_sb[tt][:], xtt_d[tt * P:(tt + 1) * P, :])

            ident = pp.tile([P, P], f16, tag="ident")
            masks.make_identity(nc, ident[:])
            ones = pp.tile([P, 65], f16, tag="ones")
            nc.gpsimd.memset(ones[:], 1.0)
            lnb = pp.tile([P, 1], f32, tag="lnb")
            nc.gpsimd.memset(lnb[:], LN64)
            epsb = pp.tile([P, 1], f32, tag="epsb")
            nc.gpsimd.memset(epsb[:], EPS)
            rsqb = pp.tile([P, 1], f32, tag="rsqb")
            nc.gpsimd.memset(rsqb[:], RSQ_B0)

            qT = pp.tile([P, 4 * T], f16, tag="qT")    # pair c at cols [c*T,(c+1)*T)
            kT = pp.tile([P, T], f16, tag="kT")        # kv0 rows 0:64, kv1 rows 64:128
            vsb = []
            for tt in range(NTT):
                vt = pp.tile([P, 130], f16, tag=f"v{tt}")
                nc.gpsimd.memset(vt[:, 64:65], 1.0)     # ones col for kv0
                nc.gpsimd.memset(vt[:, 129:130], 1.0)   # ones col for kv1
                vsb.append(vt)

            qT3 = qT[:].rearrange("p (c t) -> p c t", t=T)

            # ---------- macro-op emitters ----------
            rope_mem = {}

            def emit_p1(tt):
                pa = ps.tile([P, 1024], f32, tag="a")
                for hc in range(NHC):
                    lhs = xt_sb[tt][:, hc * P:(hc + 1) * P]
                    nc.tensor.matmul(pa[:, 0:512], lhs, wqkv_sb[hc][:, 0:512],
                                     start=(hc == 0), stop=(hc == NHC - 1))
                    nc.tensor.matmul(pa[:, 512:768], lhs, wqkv_sb[hc][:, 512:768],
                                     start=(hc == 0), stop=(hc == NHC - 1))
                # v eviction (no norm): one strided copy into both kv slots
                vt = vsb[tt]
                nc.vector.tensor_copy(
                    vt[:, 0:130].rearrange("p (s c) -> p s c", c=65)[:, :, 0:64],
                    pa[:, 640:768].rearrange("p (s c) -> p s c", c=64))
                # Evict raw q+k once to SBUF; rope the RAW values on Pool while
                # rstd is computed in parallel (rstd is a per-(t,head) scalar,
                # it commutes through RoPE), then one fused scale at the end.
                # This keeps the pa->transpose latency ~3.5us instead of ~7us.
                qev = wp.tile([P, 640], f16, tag="qev")
                nc.vector.tensor_copy(qev[:], pa[:, 0:640])
                # rstd chain: no Ln/Sqrt on ACT (keeps the single exp/square
                # table): seed y0 = exp(s0*float(bits(ms)) + b0) ~ ms^-0.5
                # within 1.5%, then one Newton step on DVE (err ~3e-4).
                sq = wp.tile([P, DQ], f32, tag="sq")
                nc.scalar.activation(sq[:], pa[:, 0:512], FT.Square)
                ksq = wp.tile([P, 128], f32, tag="ksq")
                nc.scalar.activation(ksq[:], pa[:, 512:640], FT.Square)
                red = wp.tile([P, 10], f32, tag="red")
                nc.vector.reduce_sum(red[:, 0:8].unsqueeze(-1),
                                     sq[:].rearrange("p (h d) -> p h d", d=D), axis=AX.X)
                nc.vector.reduce_sum(red[:, 8:10].unsqueeze(-1),
                                     ksq[:].rearrange("p (h d) -> p h d", d=D), axis=AX.X)
                ms = wp.tile([P, 10], f32, tag="ms")
                nc.vector.tensor_scalar(ms[:], red[:], 1.0 / D, EPS,
                                        mybir.AluOpType.mult, mybir.AluOpType.add)
                ebits = wp.tile([P, 10], f32, tag="ebits")
                nc.vector.tensor_copy(ebits[:], ms[:].bitcast(mybir.dt.int32))
                rstd = wp.tile([P, 10], f32, tag="rstd")
                nc.scalar.activation(rstd[:], ebits[:], FT.Exp, scale=RSQ_S0, bias=rsqb[:])
                ya = wp.tile([P, 10], f32, tag="ya")
                nc.vector.tensor_mul(ya[:], rstd[:], rstd[:])
                nc.vector.scalar_tensor_tensor(ya[:], ya[:], -0.5, ms[:],
                                               mybir.AluOpType.mult,
                                               mybir.AluOpType.mult)
                nc.vector.scalar_tensor_tensor(rstd[:], ya[:], 1.5, rstd[:],
                                               mybir.AluOpType.add,
                                               mybir.AluOpType.mult)
                # rope on raw q/k (SBUF-only: Pool engine)
                qe3 = qev[:, 0:512].rearrange("p (h d) -> p h d", d=D)
                ke3 = qev[:, 512:640].rearrange("p (h d) -> p h d", d=D)
                cosq = ropeq_sb[:, tt * 128:tt * 128 + 64]
                sinq = ropeq_sb[:, tt * 128 + 64:tt * 128 + 128]
                qraw = wp.tile([P, 640], f16, tag="qraw")
                qr3 = qraw[:, 0:512].rearrange("p (h d) -> p h d", d=D)
                kr3 = qraw[:, 512:640].rearrange("p (h d) -> p h d", d=D)
                tcos = wp.tile([P, DQ], f16, tag="tcos")
                nc.gpsimd.tensor_mul(tcos[:].rearrange("p (h d) -> p h d", d=D), qe3,
                                     cosq.unsqueeze(1).broadcast_to([P, NQ, D]))
                rp = wp.tile([P, DQ], f16, tag="rp")
                rp3 = rp[:].rearrange("p (h d) -> p h d", d=D)
                nc.gpsimd.tensor_mul(rp3[:, :, 0:32], qe3[:, :, 32:64],
                                     sinq[:, 0:32].unsqueeze(1).broadcast_to([P, NQ, 32]))
                nc.gpsimd.tensor_mul(rp3[:, :, 32:64], qe3[:, :, 0:32],
                                     sinq[:, 32:64].unsqueeze(1).broadcast_to([P, NQ, 32]))
                nc.gpsimd.tensor_add(qr3, tcos[:].rearrange("p (h d) -> p h d", d=D),
                                     rp3)
                cosk = ropek_sb[:, tt * 128:tt * 128 + 64]
                sink = ropek_sb[:, tt * 128 + 64:tt * 128 + 128]
                ktcos = wp.tile([P, 128], f16, tag="ktcos")
                nc.gpsimd.tensor_mul(ktcos[:].rearrange("p (h d) -> p h d", d=D), ke3,
                                     cosk.unsqueeze(1).broadcast_to([P, 2, D]))
                krp = wp.tile([P, 128], f16, tag="krp")
                krp3 = krp[:].rearrange("p (h d) -> p h d", d=D)
                nc.gpsimd.tensor_mul(krp3[:, :, 0:32], ke3[:, :, 32:64],
                                     sink[:, 0:32].unsqueeze(1).broadcast_to([P, 2, 32]))
                nc.gpsimd.tensor_mul(krp3[:, :, 32:64], ke3[:, :, 0:32],
                                     sink[:, 32:64].unsqueeze(1).broadcast_to([P, 2, 32]))
                nc.gpsimd.tensor_add(kr3, ktcos[:].rearrange("p (h d) -> p h d", d=D),
                                     krp3)
                # fused rstd scale (Pool, SBUF-only)
                qrope = wp.tile([P, DQ], f16, tag="qrope", bufs=3)
                nc.gpsimd.tensor_mul(qrope[:].rearrange("p (h d) -> p h d", d=D), qr3,
                                     rstd[:, 0:8].unsqueeze(-1).broadcast_to([P, NQ, D]))
                krope = wp.tile([P, 128], f16, tag="krope", bufs=3)
                nc.gpsimd.tensor_mul(krope[:].rearrange("p (h d) -> p h d", d=D), kr3,
                                     rstd[:, 8:10].unsqueeze(-1).broadcast_to([P, 2, D]))
                rope_mem[tt] = (qrope, krope)

            def emit_tr(tt):
                qrope, krope = rope_mem.pop(tt)
                ptr = ps.tile([P, 640], f16, tag="a")
                for c in range(4):
                    nc.tensor.transpose(ptr[:, c * P:(c + 1) * P],
                                        qrope[:, c * P:(c + 1) * P], ident[:])
                nc.tensor.transpose(ptr[:, 512:640], krope[:], ident[:])
                nc.vector.tensor_copy(qT3[:, :, tt * P:(tt + 1) * P],
                                      ptr[:, 0:512].rearrange("p (c t) -> p c t", t=P))
                nc.vector.tensor_copy(kT[:, tt * P:(tt + 1) * P], ptr[:, 512:640])

            # attention step state
            class Step:
                pass

            po_live = {}   # (i, c) -> (po_a, po_b)

            def emit_S(st):
                i, c, j = st.i, st.c, st.j
                rel = max(0, (j - 4 * i) * P)
                diag = (j >= 4 * i)
                st.rel = rel
                pa2 = ps.tile([P, 1024], f32, tag="a")
                pt = ptp.tile([P, 1024], f16, tag="pt")
                for s in range(2):
                    nc.tensor.matmul(
                        pa2[:, s * 512:(s + 1) * 512],
                        kT[s * D:(s + 1) * D, j * P:(j + 1) * P],
                        qT3[s * D:(s + 1) * D, c, i * BLK:(i + 1) * BLK],
                        start=True, stop=not diag, skip_group_check=True)
                if diag:
                    for s in range(2):
                        nc.tensor.matmul(
                            pa2[:, s * 512 + rel:s * 512 + rel + P],
                            negI, ustr, start=False, stop=True,
                            skip_group_check=True)
                nc.scalar.activation(pt[:], pa2[:], FT.Exp, scale=0.125, bias=lnb[:])
                st.pt = pt

            def emit_PV(st):
                i, c, j, rel = st.i, st.c, st.j, st.rel
                if j == 0:
                    po_live[(i, c)] = (pop.tile([65, BLK], f32, tag="po", name="po_a"),
                                       pop.tile([65, BLK], f32, tag="po", name="po_b"))
                po_a, po_b = po_live[(i, c)]
                last = 4 * (i + 1) - 1
                nc.tensor.matmul(po_a[:, rel:BLK], vsb[j][:, 0:65],
                                 st.pt[:, rel:512],
                                 start=(j == 0), stop=(j == last))
                nc.tensor.matmul(po_b[:, rel:BLK], vsb[j][:, 65:130],
                                 st.pt[:, 512 + rel:1024],
                                 start=(j == 0), stop=(j == last))
                st.pt = None

            def emit_norm(i, c, ob):
                po_a, po_b = po_live.pop((i, c))
                rinv = wp.tile([65, 1024], f32, tag="rinv")
                # custom-DVE ops silently no-op on partition-offset slices:
                # run over the full 65-partition tile, only row 64 is read
                nc.vector.reciprocal_approx_fast(rinv[0:65, 0:512], po_a[0:65, :])
                nc.vector.reciprocal_approx_fast(rinv[0:65, 512:1024], po_b[0:65, :])
                rinv16 = wp.tile([65, 1024], f16, tag="rinv16")
                nc.vector.tensor_copy(rinv16[64:65, :], rinv[64:65, :])
                pb = ps.tile([64, 1024], f32, tag="a")
                nc.tensor.matmul(pb[:, 0:512], ones[64:65, 0:64],
                                 rinv16[64:65, 0:512], start=True, stop=True)
                nc.tensor.matmul(pb[:, 512:1024], ones[64:65, 0:64],
                                 rinv16[64:65, 512:1024], start=True, stop=True)
                pbs = wp.tile([64, 1024], f16, tag="pbs")
                nc.vector.tensor_copy(pbs[:], pb[:])
                nc.vector.tensor_mul(ob[0:64, :], po_a[0:64, :], pbs[:, 0:512])
                scr = wp.tile([64, BLK], f16, tag="scr")
                nc.vector.tensor_mul(scr[:], po_b[0:64, :], pbs[:, 512:1024])
                nc.scalar.dma_start(ob[64:128, :], scr[:])

            def emit_wo(i, tl, half, obufs):
                tt = i * 4 + tl
                pA = ps.tile([P, 1024], f32, tag="a")
                for c in range(4):
                    lhs = obufs[c][:, tl * P:(tl + 1) * P]
                    for hh in range(2):
                        h4 = 2 * half + hh
                        nc.tensor.matmul(pA[:, hh * 512:(hh + 1) * 512],
                                         lhs, wo_sb[c][:, h4 * 512:(h4 + 1) * 512],
                                         start=(c == 0), stop=(c == 3))
                osb = wp.tile([P, 1024], f16, tag="osb")
                nc.vector.tensor_copy(osb[:], pA[:])
                nc.sync.dma_start(
                    out_d[tt * P:(tt + 1) * P, half * 1024:(half + 1) * 1024], osb[:])

            # ---------- global schedule ----------
            # Fillers woven into each block's attention steps. tr(t) for a
            # block's own q-tiles must land in the PREVIOUS block (its first
            # S-step reads all four q tiles).
            #  block 0: tiles 5-7 + tr(4..7)  (block 1 reads q tiles 4-7)
            #  block 1: all 8 WO halves of block 0 + tiles 8-11 + tr(8..11)
            #  block 2: tiles 12-15 + tr(12..15) + 4 WO halves of block 1
            #  block 3: rest of WO1 + all WO2
            fillers = {
                0: [("p1", 5), ("tr", 4), ("p1", 6), ("tr", 5), ("p1", 7),
                    ("tr", 6), ("p1", 8), ("tr", 7)],
                1: [("tr", 8), ("wo", 0, 0, 0), ("wo", 0, 0, 1),
                    ("p1", 9), ("wo", 0, 1, 0), ("wo", 0, 1, 1), ("tr", 9),
                    ("p1", 10), ("wo", 0, 2, 0), ("wo", 0, 2, 1), ("tr", 10),
                    ("p1", 11), ("wo", 0, 3, 0), ("wo", 0, 3, 1), ("tr", 11)],
                2: [("p1", 12), ("wo", 1, 0, 0), ("p1", 13), ("tr", 12),
                    ("wo", 1, 1, 0), ("p1", 14), ("tr", 13), ("wo", 1, 2, 0),
                    ("p1", 15), ("tr", 14), ("wo", 1, 3, 0), ("tr", 15)],
                3: [("wo", 1, tl, 1) for tl in range(4)]
                   + [("wo", 2, tl, h) for tl in range(4) for h in range(2)],
            }

            block_obufs = {i: {} for i in range(NBLK)}
            norms_done = [0] * NBLK
            pv_queue = []
            norm_queue = []

            def drain_one_pv():
                st = pv_queue.pop(0)
                emit_PV(st)
                if st.j == 4 * (st.i + 1) - 1:
                    ob = obp.tile([P, BLK], f16, tag="ob")
                    block_obufs[st.i][st.c] = ob
                    norm_queue.append((st.i, st.c, ob))

            def run_norms():
                if norm_queue:
                    i, c, ob = norm_queue.pop(0)
                    emit_norm(i, c, ob)
                    norms_done[i] += 1

            def filler_ready(ev):
                if ev[0] != "wo":
                    return True
                return norms_done[ev[1]] == 4

            def emit_filler(ev):
                if ev[0] == "p1":
                    emit_p1(ev[1])
                elif ev[0] == "tr":
                    emit_tr(ev[1])
                else:
                    _, wi, tl, h = ev
                    emit_wo(wi, tl, h, block_obufs[wi])

            # head: projection tiles 0-4 (block 0 needs 0-3), transposes deferred
            for ev in [("p1", 0), ("p1", 1), ("tr", 0), ("p1", 2), ("tr", 1),
                       ("p1", 3), ("tr", 2), ("p1", 4), ("tr", 3)]:
                emit_filler(ev)

            for i in range(NBLK):
                steps = [(c, j) for c in range(4) for j in range(4 * (i + 1))]
                fl = list(fillers[i])
                nst = len(steps)
                # one filler roughly every nst/nfl steps
                stride = max(1, nst // max(1, len(fl)))
                for n, (c, j) in enumerate(steps):
                    st = Step()
                    st.i, st.c, st.j = i, c, j
                    emit_S(st)
                    pv_queue.append(st)
                    if len(pv_queue) > L_PV:
                        drain_one_pv()
                    run_norms()
                    if fl and (n % stride == stride - 1 or n == nst - 1):
                        # pop the first ready filler; at block end flush all
                        while fl:
                            k = next((x for x in range(len(fl))
                                      if filler_ready(fl[x])), None)
                            if k is None:
                                break
                            emit_filler(fl.pop(k))
                            if n != nst - 1:
                                break
                # unready leftovers (WO waiting on this block's own norms):
                # drain pipeline enough, then emit
                while fl:
                    while pv_queue and not all(filler_ready(x) for x in fl):
                        drain_one_pv()
                        run_norms()
                    while norm_queue and not all(filler_ready(x) for x in fl):
                        run_norms()
                    k = next((x for x in range(len(fl))
                              if filler_ready(fl[x])), None)
                    if k is not None:
                        emit_filler(fl.pop(k))
                    else:
                        break
                assert not fl, f"unplaceable fillers in block {i}: {fl}"
            while pv_queue:
                drain_one_pv()
                run_norms()
            while norm_queue:
                run_norms()
            # tail: block 3 out-proj
            for tl in range(4):
                for h in range(2):
                    emit_wo(3, tl, h, block_obufs[3])

    nc.compile()
    return nc


def _host_inputs(x, Wq, Wk, Wv, Wo, q_ln_w, k_ln_w):
    x = np.asarray(x, np.float32)
    Wq = np.asarray(Wq, np.float32)
    Wk = np.asarray(Wk, np.float32)
    Wv = np.asarray(Wv, np.float32)
    Wo = np.asarray(Wo, np.float32)
    q_ln_w = np.asarray(q_ln_w, np.float64)
    k_ln_w = np.asarray(k_ln_w, np.float64)

    inv_freq = 1.0 / (1e6 ** (np.arange(0, D, 2, dtype=np.float64) / D))
    t = np.arange(T, dtype=np.float64)
    freqs = np.outer(t, inv_freq)
    emb = np.concatenate([freqs, freqs], -1)
    cos, sin = np.cos(emb), np.sin(emb)
    rot = (np.arange(D) + 32) % D
    sign = np.where(np.arange(D) < 32, -1.0, 1.0)

    def rope_tab(w):
        cw = w[None, :] * cos
        sw = sign[None, :] * w[rot][None, :] * sin
        return np.concatenate([cw, sw], -1).astype(np.float16)

    def relayout(tab):
        # [T,128] -> [128, 16*128]: row p holds tile tt's row (tt*128+p)
        return np.ascontiguousarray(
            tab.reshape(NTT, P, 128).transpose(1, 0, 2).reshape(P, NTT * 128))

    ropeq = relayout(rope_tab(q_ln_w))
    ropek = relayout(rope_tab(k_ln_w))
    pp_, gg_ = np.meshgrid(np.arange(P), np.arange(P), indexing="ij")
    negI = np.where(pp_ == gg_, MASKVAL, 0.0)
    ustr = (pp_ > gg_).astype(np.float64)
    mtab = np.concatenate([negI, ustr], axis=1).astype(np.float16)

    in_maps = []
    for core in range(8):
        b, g = core // 4, core % 4
        xT = np.ascontiguousarray(x[b].T).astype(np.float16)
        # retile so projection tile tt is one contiguous 512KB row-block:
        # xtt[tt*128+p, hc*128+t'] = xT[hc*128+p, tt*128+t']
        xtt = np.ascontiguousarray(
            xT.reshape(NHC, P, NTT, P).transpose(2, 1, 0, 3).reshape(T, H))
        heads = []
        for c in range(4):
            heads += [g * 8 + c, g * 8 + c + 4]
        wqkv = np.ascontiguousarray(np.concatenate(
            [Wq[:, h * D:(h + 1) * D] for h in heads]
            + [Wk[:, g * 128:(g + 1) * 128], Wv[:, g * 128:(g + 1) * 128]],
            axis=1)).astype(np.float16)
        wo = np.ascontiguousarray(
            np.concatenate([Wo[h * D:(h + 1) * D, :] for h in heads], axis=0)
        ).astype(np.float16)
        in_maps.append({
            "xtt": xtt, "wqkv": wqkv, "wo": wo,
            "ropeq": ropeq, "ropek": ropek, "mtab": mtab,
        })
    return in_maps


def get_program():
    if "nc" not in _CACHE:
        _CACHE["nc"] = _build_program()
    return _CACHE["nc"]


def run(inputs, trace=False, tmpdir=None):
    nc = get_program()
    in_maps = _host_inputs(**inputs)
    res = run_bass_kernel_spmd(nc, in_maps, list(range(8)), trace=trace, tmpdir=tmpdir)
    out = np.zeros((2, T, H), np.float32)
    for core in range(8):
        out[core // 4] += res.results[core]["out"].astype(np.float32)
    return out, res


def kernel(**inputs) -> np.ndarray:
    out, _ = run(inputs, trace=False)
    return out


# revision 22
# speedup vs baseline: 1.1996x; 1.1996x over previous
"""GQA attention block (qk-rmsnorm + RoPE + causal GQA attention + out-proj),
tensor-parallel over 8 NeuronCores: 2-way data parallel (batch) x 4-way head
parallel (8 q heads / 2 kv heads per core). All-reduce of out-proj partials is
done on host (sum of 4 partials per batch).

Schedule: one globally-ordered macro-op stream interleaves projection tiles,
attention steps (S -> exp -> PV per 128-key tile, both kv heads row-tiled on
the PE concurrently), per-chain softmax normalizes, and out-proj psum waves,
so the PE never idles long enough for HAM to re-throttle. ACT runs only
{Exp, Ln, Square} (one activation table, zero reloads); DVE handles all
PSUM-touching vector work; Pool (no PSUM port) gets SBUF-only rope/reduce.
"""
import sys
import numpy as np

sys.path.insert(0, "/opt/trn_rl_repo")

import concourse.bass as bass  # noqa: E402
import concourse.bacc as bacc  # noqa: E402
import concourse.mybir as mybir  # noqa: E402
import concourse.tile as tile  # noqa: E402
from concourse import masks  # noqa: E402
from concourse.bass_utils import run_bass_kernel_spmd  # noqa: E402

f32 = mybir.dt.float32
f32r = mybir.dt.float32r
f16 = mybir.dt.float16
FT = mybir.ActivationFunctionType
AX = mybir.AxisListType

P = 128
T = 2048
H = 2048
D = 64
NQ = 8          # q heads per core
DQ = NQ * D     # 512
NTT = T // P    # 16 T tiles
NHC = H // P    # 16 hidden chunks
NBLK = 4        # T_q blocks of 512
BLK = 512
EPS = 1e-5
MASKVAL = -30000.0
LN64 = -4.1588830833596715  # ln(1/64): scales exp to keep 1/rowsum in f16 normal range
L_PV = 4        # PV lag in attention steps
LN2 = 0.6931471805599453
RSQ_S0 = -0.5 * LN2 / (1 << 23)          # rsqrt seed: exp(s0*bits + b0)
RSQ_B0 = 0.5 * 127.0 * LN2 + 0.5 * 0.0430 * LN2

_CACHE = {}


def _build_program():
    nc = bacc.Bacc("TRN2", target_bir_lowering=False, debug=False, num_devices=8)

    xtt_d = nc.dram_tensor("xtt", [T, H], f16, kind="ExternalInput")
    wqkv_d = nc.dram_tensor("wqkv", [H, 768], f16, kind="ExternalInput")
    wo_d = nc.dram_tensor("wo", [DQ, H], f16, kind="ExternalInput")
    ropeq_d = nc.dram_tensor("ropeq", [P, 16 * 128], f16, kind="ExternalInput")
    ropek_d = nc.dram_tensor("ropek", [P, 16 * 128], f16, kind="ExternalInput")
    mtab_d = nc.dram_tensor("mtab", [P, 256], f16, kind="ExternalInput")
    out_d = nc.dram_tensor("out", [T, H], f16, kind="ExternalOutput")

    with tile.TileContext(nc) as tc:
        with (
            tc.tile_pool(name="persist", bufs=1) as pp,
            tc.tile_pool(name="work", bufs=2) as wp,
            tc.tile_pool(name="ptp", bufs=6) as ptp,
            tc.tile_pool(name="obp", bufs=12) as obp,
            tc.tile_pool(name="psum", bufs=2, space="PSUM") as ps,
            tc.tile_pool(name="psum_o", bufs=4, space="PSUM") as pop,
        ):
            # ---------- persistent tiles + input DMAs (arrival-ordered) ----------
            mtab = pp.tile([P, 256], f16, tag="mtab")
            nc.sync.dma_start(mtab[:], mtab_d[:])
            negI = mtab[:, 0:128]     # -30000 on diagonal
            ustr = mtab[:, 128:256]   # 1 where k > q (strict lower)

            wqkv_sb = [pp.tile([P, 768], f16, tag=f"wqkv{hc}", name=f"wqkv{hc}")
                       for hc in range(NHC)]
            xt_sb = [pp.tile([P, H], f16, tag=f"xt{tt}", name=f"xt{tt}")
                     for tt in range(NTT)]
            ropeq_sb = pp.tile([P, 16 * 128], f16, tag="ropeq")
            ropek_sb = pp.tile([P, 16 * 128], f16, tag="ropek")
            wo_sb = [pp.tile([P, H], f16, tag=f"woW{c}", name=f"woW{c}")
                     for c in range(4)]

            # weights + tables stream on the ACT HWDGE queue, x tiles on SP:
            # the two rings run in parallel and the latency-critical first
            # tile (wqkv0 + xtt0) lands in ~3us
            nc.sync.dma_start(xt_sb[0][:], xtt_d[0:P, :])
            for hc in range(NHC):
                nc.scalar.dma_start(wqkv_sb[hc][:], wqkv_d[hc * P:(hc + 1) * P, :])
            nc.scalar.dma_start(ropeq_sb[:], ropeq_d[:])
            nc.scalar.dma_start(ropek_sb[:], ropek_d[:])
            for tt in range(1, 8):
                nc.sync.dma_start(xt_sb[tt][:], xtt_d[tt * P:(tt + 1) * P, :])
            for c in range(4):
                nc.sync.dma_start(wo_sb[c][:], wo_d[c * P:(c + 1) * P, :])
            for tt in range(8, NTT):
                nc.sync.dma_start(xt_sb[tt][:], xtt_d[tt * P:(tt + 1) * P, :])

            ident = pp.tile([P, P], f16, tag="ident")
            masks.make_identity(nc, ident[:])
            ones = pp.tile([P, 65], f16, tag="ones")
            nc.gpsimd.memset(ones[:], 1.0)
            lnb = pp.tile([P, 1], f32, tag="lnb")
            nc.gpsimd.memset(lnb[:], LN64)
            epsb = pp.tile([P, 1], f32, tag="epsb")
            nc.gpsimd.memset(epsb[:], EPS)
            rsqb = pp.tile([P, 1], f32, tag="rsqb")
            nc.gpsimd.memset(rsqb[:], RSQ_B0)

            qT = pp.tile([P, 4 * T], f16, tag="qT")    # pair c at cols [c*T,(c+1)*T)
            kT = pp.tile([P, T], f16, tag="kT")        # kv0 rows 0:64, kv1 rows 64:128
            vsb = []
            for tt in range(NTT):
                vt = pp.tile([P, 130], f16, tag=f"v{tt}")
                nc.gpsimd.memset(vt[:, 64:65], 1.0)     # ones col for kv0
                nc.gpsimd.memset(vt[:, 129:130], 1.0)   # ones col for kv1
                vsb.append(vt)

            qT3 = qT[:].rearrange("p (c t) -> p c t", t=T)

            # ---------- macro-op emitters ----------
            rope_mem = {}

            def emit_p1(tt):
                pa = ps.tile([P, 1024], f32, tag="a")
                for hc in range(NHC):
                    lhs = xt_sb[tt][:, hc * P:(hc + 1) * P]
                    nc.tensor.matmul(pa[:, 0:512], lhs, wqkv_sb[hc][:, 0:512],
                                     start=(hc == 0), stop=(hc == NHC - 1))
                    nc.tensor.matmul(pa[:, 512:768], lhs, wqkv_sb[hc][:, 512:768],
                                     start=(hc == 0), stop=(hc == NHC - 1))
                # v eviction (no norm): one strided copy into both kv slots
                vt = vsb[tt]
                nc.vector.tensor_copy(
                    vt[:, 0:130].rearrange("p (s c) -> p s c", c=65)[:, :, 0:64],
                    pa[:, 640:768].rearrange("p (s c) -> p s c", c=64))
                # Evict raw q+k once to SBUF; rope the RAW values on Pool while
                # rstd is computed in parallel (rstd is a per-(t,head) scalar,
                # it commutes through RoPE), then one fused scale at the end.
                # This keeps the pa->transpose latency ~3.5us instead of ~7us.
                qev = wp.tile([P, 640], f16, tag="qev")
                nc.vector.tensor_copy(qev[:], pa[:, 0:640])
                # rstd chain: no Ln/Sqrt on ACT (keeps the single exp/square
                # table): seed y0 = exp(s0*float(bits(ms)) + b0) ~ ms^-0.5
                # within 1.5%, then one Newton step on DVE (err ~3e-4).
                sq = wp.tile([P, DQ], f32, tag="sq")
                nc.scalar.activation(sq[:], pa[:, 0:512], FT.Square)
                ksq = wp.tile([P, 128], f32, tag="ksq")
                nc.scalar.activation(ksq[:], pa[:, 512:640], FT.Square)
                red = wp.tile([P, 10], f32, tag="red")
                nc.vector.reduce_sum(red[:, 0:8].unsqueeze(-1),
                                     sq[:].rearrange("p (h d) -> p h d", d=D), axis=AX.X)
                nc.vector.reduce_sum(red[:, 8:10].unsqueeze(-1),
                                     ksq[:].rearrange("p (h d) -> p h d", d=D), axis=AX.X)
                ms = wp.tile([P, 10], f32, tag="ms")
                nc.vector.tensor_scalar(ms[:], red[:], 1.0 / D, EPS,
                                        mybir.AluOpType.mult, mybir.AluOpType.add)
                ebits = wp.tile([P, 10], f32, tag="ebits")
                nc.vector.tensor_copy(ebits[:], ms[:].bitcast(mybir.dt.int32))
                rstd = wp.tile([P, 10], f32, tag="rstd")
                nc.scalar.activation(rstd[:], ebits[:], FT.Exp, scale=RSQ_S0, bias=rsqb[:])
                ya = wp.tile([P, 10], f32, tag="ya")
                nc.vector.tensor_mul(ya[:], rstd[:], rstd[:])
                nc.vector.scalar_tensor_tensor(ya[:], ya[:], -0.5, ms[:],
                                               mybir.AluOpType.mult,
                                               mybir.AluOpType.mult)
                nc.vector.scalar_tensor_tensor(rstd[:], ya[:], 1.5, rstd[:],
                                               mybir.AluOpType.add,
                                               mybir.AluOpType.mult)
                # rope on raw q/k (SBUF-only: Pool engine)
                qe3 = qev[:, 0:512].rearrange("p (h d) -> p h d", d=D)
                ke3 = qev[:, 512:640].rearrange("p (h d) -> p h d", d=D)
                cosq = ropeq_sb[:, tt * 128:tt * 128 + 64]
                sinq = ropeq_sb[:, tt * 128 + 64:tt * 128 + 128]
                qraw = wp.tile([P, 640], f16, tag="qraw")
                qr3 = qraw[:, 0:512].rearrange("p (h d) -> p h d", d=D)
                kr3 = qraw[:, 512:640].rearrange("p (h d) -> p h d", d=D)
                tcos = wp.tile([P, DQ], f16, tag="tcos")
                nc.gpsimd.tensor_mul(tcos[:].rearrange("p (h d) -> p h d", d=D), qe3,
                                     cosq.unsqueeze(1).broadcast_to([P, NQ, D]))
                rp = wp.tile([P, DQ], f16, tag="rp")
                rp3 = rp[:].rearrange("p (h d) -> p h d", d=D)
                nc.gpsimd.tensor_mul(rp3[:, :, 0:32], qe3[:, :, 32:64],
                                     sinq[:, 0:32].unsqueeze(1).broadcast_to([P, NQ, 32]))
                nc.gpsimd.tensor_mul(rp3[:, :, 32:64], qe3[:, :, 0:32],
                                     sinq[:, 32:64].unsqueeze(1).broadcast_to([P, NQ, 32]))
                nc.gpsimd.tensor_add(qr3, tcos[:].rearrange("p (h d) -> p h d", d=D),
                                     rp3)
                cosk = ropek_sb[:, tt * 128:tt * 128 + 64]
                sink = ropek_sb[:, tt * 128 + 64:tt * 128 + 128]
                ktcos = wp.tile([P, 128], f16, tag="ktcos")
                nc.gpsimd.tensor_mul(ktcos[:].rearrange("p (h d) -> p h d", d=D), ke3,
                                     cosk.unsqueeze(1).broadcast_to([P, 2, D]))
                krp = wp.tile([P, 128], f16, tag="krp")
                krp3 = krp[:].rearrange("p (h d) -> p h d", d=D)
                nc.gpsimd.tensor_mul(krp3[:, :, 0:32], ke3[:, :, 32:64],
                                     sink[:, 0:32].unsqueeze(1).broadcast_to([P, 2, 32]))
                nc.gpsimd.tensor_mul(krp3[:, :, 32:64], ke3[:, :, 0:32],
                                     sink[:, 32:64].unsqueeze(1).broadcast_to([P, 2, 32]))
                nc.gpsimd.tensor_add(kr3, ktcos[:].rearrange("p (h d) -> p h d", d=D),
                                     krp3)
                # fused rstd scale (Pool, SBUF-only)
                qrope = wp.tile([P, DQ], f16, tag="qrope", bufs=3)
                nc.gpsimd.tensor_mul(qrope[:].rearrange("p (h d) -> p h d", d=D), qr3,
                                     rstd[:, 0:8].unsqueeze(-1).broadcast_to([P, NQ, D]))
                krope = wp.tile([P, 128], f16, tag="krope", bufs=3)
                nc.gpsimd.tensor_mul(krope[:].rearrange("p (h d) -> p h d", d=D), kr3,
                                     rstd[:, 8:10].unsqueeze(-1).broadcast_to([P, 2, D]))
                rope_mem[tt] = (qrope, krope)

            def emit_tr(tt):
                qrope, krope = rope_mem.pop(tt)
                ptr = ps.tile([P, 640], f16, tag="a")
                for c in range(4):
                    nc.tensor.transpose(ptr[:, c * P:(c + 1) * P],
                                        qrope[:, c * P:(c + 1) * P], ident[:])
                nc.tensor.transpose(ptr[:, 512:640], krope[:], ident[:])
                nc.vector.tensor_copy(qT3[:, :, tt * P:(tt + 1) * P],
                                      ptr[:, 0:512].rearrange("p (c t) -> p c t", t=P))
                nc.vector.tensor_copy(kT[:, tt * P:(tt + 1) * P], ptr[:, 512:640])

            # attention step state
            class Step:
                pass

            po_live = {}   # (i, c) -> (po_a, po_b)

            def emit_S(st):
                i, c, j = st.i, st.c, st.j
                rel = max(0, (j - 4 * i) * P)
                diag = (j >= 4 * i)
                st.rel = rel
                pa2 = ps.tile([P, 1024], f32, tag="a")
                pt = ptp.tile([P, 1024], f16, tag="pt")
                for s in range(2):
                    nc.tensor.matmul(
                        pa2[:, s * 512:(s + 1) * 512],
                        kT[s * D:(s + 1) * D, j * P:(j + 1) * P],
                        qT3[s * D:(s + 1) * D, c, i * BLK:(i + 1) * BLK],
                        start=True, stop=not diag, skip_group_check=True)
                if diag:
                    for s in range(2):
                        nc.tensor.matmul(
                            pa2[:, s * 512 + rel:s * 512 + rel + P],
                            negI, ustr, start=False, stop=True,
                            skip_group_check=True)
                nc.scalar.activation(pt[:], pa2[:], FT.Exp, scale=0.125, bias=lnb[:])
                st.pt = pt

            def emit_PV(st):
                i, c, j, rel = st.i, st.c, st.j, st.rel
                if j == 0:
                    po_live[(i, c)] = (pop.tile([65, BLK], f32, tag="po", name="po_a"),
                                       pop.tile([65, BLK], f32, tag="po", name="po_b"))
                po_a, po_b = po_live[(i, c)]
                last = 4 * (i + 1) - 1
                nc.tensor.matmul(po_a[:, rel:BLK], vsb[j][:, 0:65],
                                 st.pt[:, rel:512],
                                 start=(j == 0), stop=(j == last))
                nc.tensor.matmul(po_b[:, rel:BLK], vsb[j][:, 65:130],
                                 st.pt[:, 512 + rel:1024],
                                 start=(j == 0), stop=(j == last))
                st.pt = None

            def emit_norm(i, c, ob):
                po_a, po_b = po_live.pop((i, c))
                rinv = wp.tile([65, 1024], f32, tag="rinv")
                # custom-DVE ops silently no-op on partition-offset slices:
                # run over the full 65-partition tile, only row 64 is read
                nc.vector.reciprocal_approx_fast(rinv[0:65, 0:512], po_a[0:65, :])
                nc.vector.reciprocal_approx_fast(rinv[0:65, 512:1024], po_b[0:65, :])
                rinv16 = wp.tile([65, 1024], f16, tag="rinv16")
                nc.vector.tensor_copy(rinv16[64:65, :], rinv[64:65, :])
                pb = ps.tile([64, 1024], f32, tag="a")
                nc.tensor.matmul(pb[:, 0:512], ones[64:65, 0:64],
                                 rinv16[64:65, 0:512], start=True, stop=True)
                nc.tensor.matmul(pb[:, 512:1024], ones[64:65, 0:64],
                                 rinv16[64:65, 512:1024], start=True, stop=True)
                pbs = wp.tile([64, 1024], f16, tag="pbs")
                nc.vector.tensor_copy(pbs[:], pb[:])
                nc.vector.tensor_mul(ob[0:64, :], po_a[0:64, :], pbs[:, 0:512])
                scr = wp.tile([64, BLK], f16, tag="scr")
                nc.vector.tensor_mul(scr[:], po_b[0:64, :], pbs[:, 512:1024])
                nc.sync.dma_start(ob[64:128, :], scr[:])

            def emit_wo(i, tl, half, obufs):
                tt = i * 4 + tl
                pA = ps.tile([P, 1024], f32, tag="a")
                for c in range(4):
                    lhs = obufs[c][:, tl * P:(tl + 1) * P]
                    for hh in range(2):
                        h4 = 2 * half + hh
                        nc.tensor.matmul(pA[:, hh * 512:(hh + 1) * 512],
                                         lhs, wo_sb[c][:, h4 * 512:(h4 + 1) * 512],
                                         start=(c == 0), stop=(c == 3))
                osb = wp.tile([P, 1024], f16, tag="osb")
                nc.vector.tensor_copy(osb[:], pA[:])
                nc.sync.dma_start(
                    out_d[tt * P:(tt + 1) * P, half * 1024:(half + 1) * 1024], osb[:])

            # ---------- global schedule ----------
            # Fillers woven into each block's attention steps. tr(t) for a
            # block's own q-tiles must land in the PREVIOUS block (its first
            # S-step reads all four q tiles).
            #  block 0: tiles 5-7 + tr(4..7)  (block 1 reads q tiles 4-7)
            #  block 1: all 8 WO halves of block 0 + tiles 8-11 + tr(8..11)
            #  block 2: tiles 12-15 + tr(12..15) + 4 WO halves of block 1
            #  block 3: rest of WO1 + all WO2
            fillers = {
                0: [("p1", 5), ("p1", 6), ("tr", 4), ("tr", 5), ("p1", 7),
                    ("tr", 6), ("p1", 8), ("tr", 7)],
                1: [("tr", 8), ("wo", 0, 0, 0), ("wo", 0, 0, 1),
                    ("p1", 9), ("wo", 0, 1, 0), ("wo", 0, 1, 1), ("tr", 9),
                    ("p1", 10), ("wo", 0, 2, 0), ("wo", 0, 2, 1), ("tr", 10),
                    ("p1", 11), ("wo", 0, 3, 0), ("wo", 0, 3, 1), ("tr", 11)],
                2: [("p1", 12), ("wo", 1, 0, 0), ("p1", 13), ("tr", 12),
                    ("wo", 1, 1, 0), ("p1", 14), ("tr", 13), ("wo", 1, 2, 0),
                    ("p1", 15), ("tr", 14), ("wo", 1, 3, 0), ("tr", 15)],
                3: [("wo", 1, tl, 1) for tl in range(4)]
                   + [("wo", 2, tl, h) for tl in range(4) for h in range(2)],
            }

            block_obufs = {i: {} for i in range(NBLK)}
            norms_done = [0] * NBLK
            pv_queue = []
            norm_queue = []

            def drain_one_pv():
                st = pv_queue.pop(0)
                emit_PV(st)
                if st.j == 4 * (st.i + 1) - 1:
                    ob = obp.tile([P, BLK], f16, tag="ob")
                    block_obufs[st.i][st.c] = ob
                    norm_queue.append((st.i, st.c, ob))

            def run_norms():
                if norm_queue:
                    i, c, ob = norm_queue.pop(0)
                    emit_norm(i, c, ob)
                    norms_done[i] += 1

            def filler_ready(ev):
                if ev[0] != "wo":
                    return True
                return norms_done[ev[1]] == 4

            def emit_filler(ev):
                if ev[0] == "p1":
                    emit_p1(ev[1])
                elif ev[0] == "tr":
                    emit_tr(ev[1])
                else:
                    _, wi, tl, h = ev
                    emit_wo(wi, tl, h, block_obufs[wi])

            # head: projection tiles 0-4 (block 0 needs 0-3), transposes deferred
            for ev in [("p1", 0), ("p1", 1), ("tr", 0), ("p1", 2), ("tr", 1),
                       ("p1", 3), ("tr", 2), ("p1", 4), ("tr", 3)]:
                emit_filler(ev)

            for i in range(NBLK):
                steps = [(c, j) for c in range(4) for j in range(4 * (i + 1))]
                fl = list(fillers[i])
                nst = len(steps)
                # one filler roughly every nst/nfl steps
                stride = max(1, nst // max(1, len(fl)))
                for n, (c, j) in enumerate(steps):
                    st = Step()
                    st.i, st.c, st.j = i, c, j
                    emit_S(st)
                    pv_queue.append(st)
                    if len(pv_queue) > L_PV:
                        drain_one_pv()
                    run_norms()
                    if fl and (n % stride == stride - 1 or n == nst - 1):
                        # pop the first ready filler; at block end flush all
                        while fl:
                            k = next((x for x in range(len(fl))
                                      if filler_ready(fl[x])), None)
                            if k is None:
                                break
                            emit_filler(fl.pop(k))
                            if n != nst - 1:
                                break
                # unready leftovers (WO waiting on this block's own norms):
                # drain pipeline enough, then emit
                while fl:
                    while pv_queue and not all(filler_ready(x) for x in fl):
                        drain_one_pv()
                        run_norms()
                    while norm_queue and not all(filler_ready(x) for x in fl):
                        run_norms()
                    k = next((x for x in range(len(fl))
                              if filler_ready(fl[x])), None)
                    if k is not None:
                        emit_filler(fl.pop(k))
                    else:
                        break
                assert not fl, f"unplaceable fillers in block {i}: {fl}"
            while pv_queue:
                drain_one_pv()
                run_norms()
            while norm_queue:
                run_norms()
            # tail: block 3 out-proj
            for tl in range(4):
                for h in range(2):
                    emit_wo(3, tl, h, block_obufs[3])

    nc.compile()
    return nc


def _host_inputs(x, Wq, Wk, Wv, Wo, q_ln_w, k_ln_w):
    x = np.asarray(x, np.float32)
    Wq = np.asarray(Wq, np.float32)
    Wk = np.asarray(Wk, np.float32)
    Wv = np.asarray(Wv, np.float32)
    Wo = np.asarray(Wo, np.float32)
    q_ln_w = np.asarray(q_ln_w, np.float64)
    k_ln_w = np.asarray(k_ln_w, np.float64)

    inv_freq = 1.0 / (1e6 ** (np.arange(0, D, 2, dtype=np.float64) / D))
    t = np.arange(T, dtype=np.float64)
    freqs = np.outer(t, inv_freq)
    emb = np.concatenate([freqs, freqs], -1)
    cos, sin = np.cos(emb), np.sin(emb)
    rot = (np.arange(D) + 32) % D
    sign = np.where(np.arange(D) < 32, -1.0, 1.0)

    def rope_tab(w):
        cw = w[None, :] * cos
        sw = sign[None, :] * w[rot][None, :] * sin
        return np.concatenate([cw, sw], -1).astype(np.float16)

    def relayout(tab):
        # [T,128] -> [128, 16*128]: row p holds tile tt's row (tt*128+p)
        return np.ascontiguousarray(
            tab.reshape(NTT, P, 128).transpose(1, 0, 2).reshape(P, NTT * 128))

    ropeq = relayout(rope_tab(q_ln_w))
    ropek = relayout(rope_tab(k_ln_w))
    pp_, gg_ = np.meshgrid(np.arange(P), np.arange(P), indexing="ij")
    negI = np.where(pp_ == gg_, MASKVAL, 0.0)
    ustr = (pp_ > gg_).astype(np.float64)
    mtab = np.concatenate([negI, ustr], axis=1).astype(np.float16)

    in_maps = []
    for core in range(8):
        b, g = core // 4, core % 4
        xT = np.ascontiguousarray(x[b].T).astype(np.float16)
        # retile so projection tile tt is one contiguous 512KB row-block:
        # xtt[tt*128+p, hc*128+t'] = xT[hc*128+p, tt*128+t']
        xtt = np.ascontiguousarray(
            xT.reshape(NHC, P, NTT, P).transpose(2, 1, 0, 3).reshape(T, H))
        heads = []
        for c in range(4):
            heads += [g * 8 + c, g * 8 + c + 4]
        wqkv = np.ascontiguousarray(np.concatenate(
            [Wq[:, h * D:(h + 1) * D] for h in heads]
            + [Wk[:, g * 128:(g + 1) * 128], Wv[:, g * 128:(g + 1) * 128]],
            axis=1)).astype(np.float16)
        wo = np.ascontiguousarray(
            np.concatenate([Wo[h * D:(h + 1) * D, :] for h in heads], axis=0)
        ).astype(np.float16)
        in_maps.append({
            "xtt": xtt, "wqkv": wqkv, "wo": wo,
            "ropeq": ropeq, "ropek": ropek, "mtab": mtab,
        })
    return in_maps


def get_program():
    if "nc" not in _CACHE:
        _CACHE["nc"] = _build_program()
    return _CACHE["nc"]


def run(inputs, trace=False, tmpdir=None):
    nc = get_program()
    in_maps = _host_inputs(**inputs)
    res = run_bass_kernel_spmd(nc, in_maps, list(range(8)), trace=trace, tmpdir=tmpdir)
    out = np.zeros((2, T, H), np.float32)
    for core in range(8):
        out[core // 4] += res.results[core]["out"].astype(np.float32)
    return out, res


def kernel(**inputs) -> np.ndarray:
    out, _ = run(inputs, trace=False)
    return out


# revision 23
# speedup vs baseline: 1.2055x; 1.0049x over previous
"""GQA attention block (qk-rmsnorm + RoPE + causal GQA attention + out-proj),
tensor-parallel over 8 NeuronCores: 2-way data parallel (batch) x 4-way head
parallel (8 q heads / 2 kv heads per core). All-reduce of out-proj partials is
done on host (sum of 4 partials per batch).

Schedule: one globally-ordered macro-op stream interleaves projection tiles,
attention steps (S -> exp -> PV per 128-key tile, both kv heads row-tiled on
the PE concurrently), per-chain softmax normalizes, and out-proj psum waves,
so the PE never idles long enough for HAM to re-throttle. ACT runs only
{Exp, Ln, Square} (one activation table, zero reloads); DVE handles all
PSUM-touching vector work; Pool (no PSUM port) gets SBUF-only rope/reduce.
"""
import sys
import numpy as np

sys.path.insert(0, "/opt/trn_rl_repo")

import concourse.bass as bass  # noqa: E402
import concourse.bacc as bacc  # noqa: E402
import concourse.mybir as mybir  # noqa: E402
import concourse.tile as tile  # noqa: E402
from concourse import masks  # noqa: E402
from concourse.bass_utils import run_bass_kernel_spmd  # noqa: E402

f32 = mybir.dt.float32
f32r = mybir.dt.float32r
f16 = mybir.dt.float16
FT = mybir.ActivationFunctionType
AX = mybir.AxisListType

P = 128
T = 2048
H = 2048
D = 64
NQ = 8          # q heads per core
DQ = NQ * D     # 512
NTT = T // P    # 16 T tiles
NHC = H // P    # 16 hidden chunks
NBLK = 4        # T_q blocks of 512
BLK = 512
EPS = 1e-5
MASKVAL = -30000.0
LN64 = -4.1588830833596715  # ln(1/64): scales exp to keep 1/rowsum in f16 normal range
L_PV = 4        # PV lag in attention steps
LN2 = 0.6931471805599453
RSQ_S0 = -0.5 * LN2 / (1 << 23)          # rsqrt seed: exp(s0*bits + b0)
RSQ_B0 = 0.5 * 127.0 * LN2 + 0.5 * 0.0430 * LN2

_CACHE = {}


def _build_program():
    nc = bacc.Bacc("TRN2", target_bir_lowering=False, debug=False, num_devices=8)

    xtt_d = nc.dram_tensor("xtt", [T, H], f16, kind="ExternalInput")
    wqkv_d = nc.dram_tensor("wqkv", [H, 768], f16, kind="ExternalInput")
    wo_d = nc.dram_tensor("wo", [DQ, H], f16, kind="ExternalInput")
    ropeq_d = nc.dram_tensor("ropeq", [P, 16 * 128], f16, kind="ExternalInput")
    ropek_d = nc.dram_tensor("ropek", [P, 16 * 128], f16, kind="ExternalInput")
    mtab_d = nc.dram_tensor("mtab", [P, 256], f16, kind="ExternalInput")
    out_d = nc.dram_tensor("out", [T, H], f16, kind="ExternalOutput")

    with tile.TileContext(nc) as tc:
        with (
            tc.tile_pool(name="persist", bufs=1) as pp,
            tc.tile_pool(name="work", bufs=2) as wp,
            tc.tile_pool(name="ptp", bufs=6) as ptp,
            tc.tile_pool(name="obp", bufs=12) as obp,
            tc.tile_pool(name="psum", bufs=2, space="PSUM") as ps,
            tc.tile_pool(name="psum_o", bufs=4, space="PSUM") as pop,
        ):
            # ---------- persistent tiles + input DMAs (arrival-ordered) ----------
            mtab = pp.tile([P, 256], f16, tag="mtab")
            nc.sync.dma_start(mtab[:], mtab_d[:])
            negI = mtab[:, 0:128]     # -30000 on diagonal
            ustr = mtab[:, 128:256]   # 1 where k > q (strict lower)

            wqkv_sb = [pp.tile([P, 768], f16, tag=f"wqkv{hc}", name=f"wqkv{hc}")
                       for hc in range(NHC)]
            xt_sb = [pp.tile([P, H], f16, tag=f"xt{tt}", name=f"xt{tt}")
                     for tt in range(NTT)]
            ropeq_sb = pp.tile([P, 16 * 128], f16, tag="ropeq")
            ropek_sb = pp.tile([P, 16 * 128], f16, tag="ropek")
            wo_sb = [pp.tile([P, H], f16, tag=f"woW{c}", name=f"woW{c}")
                     for c in range(4)]

            # weights + tables stream on the ACT HWDGE queue, x tiles on SP:
            # the two rings run in parallel and the latency-critical first
            # tile (wqkv0 + xtt0) lands in ~3us
            nc.sync.dma_start(xt_sb[0][:], xtt_d[0:P, :])
            for hc in range(0, NHC, 2):
                nc.scalar.dma_start(wqkv_sb[hc][:], wqkv_d[hc * P:(hc + 1) * P, :])
            for hc in range(1, NHC, 2):
                nc.sync.dma_start(wqkv_sb[hc][:], wqkv_d[hc * P:(hc + 1) * P, :])
            nc.scalar.dma_start(ropeq_sb[:], ropeq_d[:])
            nc.scalar.dma_start(ropek_sb[:], ropek_d[:])
            for tt in range(1, 8):
                nc.sync.dma_start(xt_sb[tt][:], xtt_d[tt * P:(tt + 1) * P, :])
            for c in range(4):
                nc.sync.dma_start(wo_sb[c][:], wo_d[c * P:(c + 1) * P, :])
            for tt in range(8, NTT):
                nc.sync.dma_start(xt_sb[tt][:], xtt_d[tt * P:(tt + 1) * P, :])

            ident = pp.tile([P, P], f16, tag="ident")
            masks.make_identity(nc, ident[:])
            ones = pp.tile([P, 65], f16, tag="ones")
            nc.gpsimd.memset(ones[:], 1.0)
            lnb = pp.tile([P, 1], f32, tag="lnb")
            nc.gpsimd.memset(lnb[:], LN64)
            epsb = pp.tile([P, 1], f32, tag="epsb")
            nc.gpsimd.memset(epsb[:], EPS)
            rsqb = pp.tile([P, 1], f32, tag="rsqb")
            nc.gpsimd.memset(rsqb[:], RSQ_B0)

            qT = pp.tile([P, 4 * T], f16, tag="qT")    # pair c at cols [c*T,(c+1)*T)
            kT = pp.tile([P, T], f16, tag="kT")        # kv0 rows 0:64, kv1 rows 64:128
            vsb = []
            for tt in range(NTT):
                vt = pp.tile([P, 130], f16, tag=f"v{tt}")
                nc.gpsimd.memset(vt[:, 64:65], 1.0)     # ones col for kv0
                nc.gpsimd.memset(vt[:, 129:130], 1.0)   # ones col for kv1
                vsb.append(vt)

            qT3 = qT[:].rearrange("p (c t) -> p c t", t=T)

            # ---------- macro-op emitters ----------
            rope_mem = {}

            def emit_p1(tt):
                pa = ps.tile([P, 1024], f32, tag="a")
                for hc in range(NHC):
                    lhs = xt_sb[tt][:, hc * P:(hc + 1) * P]
                    nc.tensor.matmul(pa[:, 0:512], lhs, wqkv_sb[hc][:, 0:512],
                                     start=(hc == 0), stop=(hc == NHC - 1))
                    nc.tensor.matmul(pa[:, 512:768], lhs, wqkv_sb[hc][:, 512:768],
                                     start=(hc == 0), stop=(hc == NHC - 1))
                # v eviction (no norm): one strided copy into both kv slots
                vt = vsb[tt]
                nc.vector.tensor_copy(
                    vt[:, 0:130].rearrange("p (s c) -> p s c", c=65)[:, :, 0:64],
                    pa[:, 640:768].rearrange("p (s c) -> p s c", c=64))
                # Evict raw q+k once to SBUF; rope the RAW values on Pool while
                # rstd is computed in parallel (rstd is a per-(t,head) scalar,
                # it commutes through RoPE), then one fused scale at the end.
                # This keeps the pa->transpose latency ~3.5us instead of ~7us.
                qev = wp.tile([P, 640], f16, tag="qev")
                nc.vector.tensor_copy(qev[:], pa[:, 0:640])
                # rstd chain: no Ln/Sqrt on ACT (keeps the single exp/square
                # table): seed y0 = exp(s0*float(bits(ms)) + b0) ~ ms^-0.5
                # within 1.5%, then one Newton step on DVE (err ~3e-4).
                sq = wp.tile([P, DQ], f32, tag="sq")
                nc.scalar.activation(sq[:], pa[:, 0:512], FT.Square)
                ksq = wp.tile([P, 128], f32, tag="ksq")
                nc.scalar.activation(ksq[:], pa[:, 512:640], FT.Square)
                red = wp.tile([P, 10], f32, tag="red")
                nc.vector.reduce_sum(red[:, 0:8].unsqueeze(-1),
                                     sq[:].rearrange("p (h d) -> p h d", d=D), axis=AX.X)
                nc.vector.reduce_sum(red[:, 8:10].unsqueeze(-1),
                                     ksq[:].rearrange("p (h d) -> p h d", d=D), axis=AX.X)
                ms = wp.tile([P, 10], f32, tag="ms")
                nc.vector.tensor_scalar(ms[:], red[:], 1.0 / D, EPS,
                                        mybir.AluOpType.mult, mybir.AluOpType.add)
                ebits = wp.tile([P, 10], f32, tag="ebits")
                nc.vector.tensor_copy(ebits[:], ms[:].bitcast(mybir.dt.int32))
                rstd = wp.tile([P, 10], f32, tag="rstd")
                nc.scalar.activation(rstd[:], ebits[:], FT.Exp, scale=RSQ_S0, bias=rsqb[:])
                ya = wp.tile([P, 10], f32, tag="ya")
                nc.vector.tensor_mul(ya[:], rstd[:], rstd[:])
                nc.vector.scalar_tensor_tensor(ya[:], ya[:], -0.5, ms[:],
                                               mybir.AluOpType.mult,
                                               mybir.AluOpType.mult)
                nc.vector.scalar_tensor_tensor(rstd[:], ya[:], 1.5, rstd[:],
                                               mybir.AluOpType.add,
                                               mybir.AluOpType.mult)
                # rope on raw q/k (SBUF-only: Pool engine)
                qe3 = qev[:, 0:512].rearrange("p (h d) -> p h d", d=D)
                ke3 = qev[:, 512:640].rearrange("p (h d) -> p h d", d=D)
                cosq = ropeq_sb[:, tt * 128:tt * 128 + 64]
                sinq = ropeq_sb[:, tt * 128 + 64:tt * 128 + 128]
                qraw = wp.tile([P, 640], f16, tag="qraw")
                qr3 = qraw[:, 0:512].rearrange("p (h d) -> p h d", d=D)
                kr3 = qraw[:, 512:640].rearrange("p (h d) -> p h d", d=D)
                tcos = wp.tile([P, DQ], f16, tag="tcos")
                nc.gpsimd.tensor_mul(tcos[:].rearrange("p (h d) -> p h d", d=D), qe3,
                                     cosq.unsqueeze(1).broadcast_to([P, NQ, D]))
                rp = wp.tile([P, DQ], f16, tag="rp")
                rp3 = rp[:].rearrange("p (h d) -> p h d", d=D)
                nc.gpsimd.tensor_mul(rp3[:, :, 0:32], qe3[:, :, 32:64],
                                     sinq[:, 0:32].unsqueeze(1).broadcast_to([P, NQ, 32]))
                nc.gpsimd.tensor_mul(rp3[:, :, 32:64], qe3[:, :, 0:32],
                                     sinq[:, 32:64].unsqueeze(1).broadcast_to([P, NQ, 32]))
                nc.gpsimd.tensor_add(qr3, tcos[:].rearrange("p (h d) -> p h d", d=D),
                                     rp3)
                cosk = ropek_sb[:, tt * 128:tt * 128 + 64]
                sink = ropek_sb[:, tt * 128 + 64:tt * 128 + 128]
                ktcos = wp.tile([P, 128], f16, tag="ktcos")
                nc.gpsimd.tensor_mul(ktcos[:].rearrange("p (h d) -> p h d", d=D), ke3,
                                     cosk.unsqueeze(1).broadcast_to([P, 2, D]))
                krp = wp.tile([P, 128], f16, tag="krp")
                krp3 = krp[:].rearrange("p (h d) -> p h d", d=D)
                nc.gpsimd.tensor_mul(krp3[:, :, 0:32], ke3[:, :, 32:64],
                                     sink[:, 0:32].unsqueeze(1).broadcast_to([P, 2, 32]))
                nc.gpsimd.tensor_mul(krp3[:, :, 32:64], ke3[:, :, 0:32],
                                     sink[:, 32:64].unsqueeze(1).broadcast_to([P, 2, 32]))
                nc.gpsimd.tensor_add(kr3, ktcos[:].rearrange("p (h d) -> p h d", d=D),
                                     krp3)
                # fused rstd scale (Pool, SBUF-only)
                qrope = wp.tile([P, DQ], f16, tag="qrope", bufs=3)
                nc.gpsimd.tensor_mul(qrope[:].rearrange("p (h d) -> p h d", d=D), qr3,
                                     rstd[:, 0:8].unsqueeze(-1).broadcast_to([P, NQ, D]))
                krope = wp.tile([P, 128], f16, tag="krope", bufs=3)
                nc.gpsimd.tensor_mul(krope[:].rearrange("p (h d) -> p h d", d=D), kr3,
                                     rstd[:, 8:10].unsqueeze(-1).broadcast_to([P, 2, D]))
                rope_mem[tt] = (qrope, krope)

            def emit_tr(tt):
                qrope, krope = rope_mem.pop(tt)
                ptr = ps.tile([P, 640], f16, tag="a")
                for c in range(4):
                    nc.tensor.transpose(ptr[:, c * P:(c + 1) * P],
                                        qrope[:, c * P:(c + 1) * P], ident[:])
                nc.tensor.transpose(ptr[:, 512:640], krope[:], ident[:])
                nc.vector.tensor_copy(qT3[:, :, tt * P:(tt + 1) * P],
                                      ptr[:, 0:512].rearrange("p (c t) -> p c t", t=P))
                nc.vector.tensor_copy(kT[:, tt * P:(tt + 1) * P], ptr[:, 512:640])

            # attention step state
            class Step:
                pass

            po_live = {}   # (i, c) -> (po_a, po_b)

            def emit_S(st):
                i, c, j = st.i, st.c, st.j
                rel = max(0, (j - 4 * i) * P)
                diag = (j >= 4 * i)
                st.rel = rel
                pa2 = ps.tile([P, 1024], f32, tag="a")
                pt = ptp.tile([P, 1024], f16, tag="pt")
                for s in range(2):
                    nc.tensor.matmul(
                        pa2[:, s * 512:(s + 1) * 512],
                        kT[s * D:(s + 1) * D, j * P:(j + 1) * P],
                        qT3[s * D:(s + 1) * D, c, i * BLK:(i + 1) * BLK],
                        start=True, stop=not diag, skip_group_check=True)
                if diag:
                    for s in range(2):
                        nc.tensor.matmul(
                            pa2[:, s * 512 + rel:s * 512 + rel + P],
                            negI, ustr, start=False, stop=True,
                            skip_group_check=True)
                nc.scalar.activation(pt[:], pa2[:], FT.Exp, scale=0.125, bias=lnb[:])
                st.pt = pt

            def emit_PV(st):
                i, c, j, rel = st.i, st.c, st.j, st.rel
                if j == 0:
                    po_live[(i, c)] = (pop.tile([65, BLK], f32, tag="po", name="po_a"),
                                       pop.tile([65, BLK], f32, tag="po", name="po_b"))
                po_a, po_b = po_live[(i, c)]
                last = 4 * (i + 1) - 1
                nc.tensor.matmul(po_a[:, rel:BLK], vsb[j][:, 0:65],
                                 st.pt[:, rel:512],
                                 start=(j == 0), stop=(j == last))
                nc.tensor.matmul(po_b[:, rel:BLK], vsb[j][:, 65:130],
                                 st.pt[:, 512 + rel:1024],
                                 start=(j == 0), stop=(j == last))
                st.pt = None

            def emit_norm(i, c, ob):
                po_a, po_b = po_live.pop((i, c))
                rinv = wp.tile([65, 1024], f32, tag="rinv")
                # custom-DVE ops silently no-op on partition-offset slices:
                # run over the full 65-partition tile, only row 64 is read
                nc.vector.reciprocal_approx_fast(rinv[0:65, 0:512], po_a[0:65, :])
                nc.vector.reciprocal_approx_fast(rinv[0:65, 512:1024], po_b[0:65, :])
                rinv16 = wp.tile([65, 1024], f16, tag="rinv16")
                nc.vector.tensor_copy(rinv16[64:65, :], rinv[64:65, :])
                pb = ps.tile([64, 1024], f32, tag="a")
                nc.tensor.matmul(pb[:, 0:512], ones[64:65, 0:64],
                                 rinv16[64:65, 0:512], start=True, stop=True)
                nc.tensor.matmul(pb[:, 512:1024], ones[64:65, 0:64],
                                 rinv16[64:65, 512:1024], start=True, stop=True)
                pbs = wp.tile([64, 1024], f16, tag="pbs")
                nc.vector.tensor_copy(pbs[:], pb[:])
                nc.vector.tensor_mul(ob[0:64, :], po_a[0:64, :], pbs[:, 0:512])
                scr = wp.tile([64, BLK], f16, tag="scr")
                nc.vector.tensor_mul(scr[:], po_b[0:64, :], pbs[:, 512:1024])
                nc.sync.dma_start(ob[64:128, :], scr[:])

            def emit_wo(i, tl, half, obufs):
                tt = i * 4 + tl
                pA = ps.tile([P, 1024], f32, tag="a")
                for c in range(4):
                    lhs = obufs[c][:, tl * P:(tl + 1) * P]
                    for hh in range(2):
                        h4 = 2 * half + hh
                        nc.tensor.matmul(pA[:, hh * 512:(hh + 1) * 512],
                                         lhs, wo_sb[c][:, h4 * 512:(h4 + 1) * 512],
                                         start=(c == 0), stop=(c == 3))
                osb = wp.tile([P, 1024], f16, tag="osb")
                nc.vector.tensor_copy(osb[:], pA[:])
                nc.sync.dma_start(
                    out_d[tt * P:(tt + 1) * P, half * 1024:(half + 1) * 1024], osb[:])

            # ---------- global schedule ----------
            # Fillers woven into each block's attention steps. tr(t) for a
            # block's own q-tiles must land in the PREVIOUS block (its first
            # S-step reads all four q tiles).
            #  block 0: tiles 5-7 + tr(4..7)  (block 1 reads q tiles 4-7)
            #  block 1: all 8 WO halves of block 0 + tiles 8-11 + tr(8..11)
            #  block 2: tiles 12-15 + tr(12..15) + 4 WO halves of block 1
            #  block 3: rest of WO1 + all WO2
            fillers = {
                0: [("p1", 5), ("p1", 6), ("tr", 4), ("tr", 5), ("p1", 7),
                    ("tr", 6), ("p1", 8), ("tr", 7)],
                1: [("tr", 8), ("p1", 9), ("tr", 9), ("wo", 0, 0, 0),
                    ("wo", 0, 0, 1), ("p1", 10), ("wo", 0, 1, 0),
                    ("wo", 0, 1, 1), ("tr", 10), ("p1", 11), ("wo", 0, 2, 0),
                    ("wo", 0, 2, 1), ("tr", 11), ("wo", 0, 3, 0),
                    ("wo", 0, 3, 1)],
                2: [("p1", 12), ("p1", 13), ("tr", 12), ("wo", 1, 0, 0),
                    ("p1", 14), ("tr", 13), ("wo", 1, 1, 0), ("p1", 15),
                    ("tr", 14), ("wo", 1, 2, 0), ("tr", 15), ("wo", 1, 3, 0)],
                3: [("wo", 1, tl, 1) for tl in range(4)]
                   + [("wo", 2, tl, h) for tl in range(4) for h in range(2)],
            }

            block_obufs = {i: {} for i in range(NBLK)}
            norms_done = [0] * NBLK
            pv_queue = []
            norm_queue = []

            def drain_one_pv():
                st = pv_queue.pop(0)
                emit_PV(st)
                if st.j == 4 * (st.i + 1) - 1:
                    ob = obp.tile([P, BLK], f16, tag="ob")
                    block_obufs[st.i][st.c] = ob
                    norm_queue.append((st.i, st.c, ob))

            def run_norms():
                if norm_queue:
                    i, c, ob = norm_queue.pop(0)
                    emit_norm(i, c, ob)
                    norms_done[i] += 1

            def filler_ready(ev):
                if ev[0] != "wo":
                    return True
                return norms_done[ev[1]] == 4

            def emit_filler(ev):
                if ev[0] == "p1":
                    emit_p1(ev[1])
                elif ev[0] == "tr":
                    emit_tr(ev[1])
                else:
                    _, wi, tl, h = ev
                    emit_wo(wi, tl, h, block_obufs[wi])

            # head: projection tiles 0-4 (block 0 needs 0-3), transposes deferred
            for ev in [("p1", 0), ("p1", 1), ("tr", 0), ("p1", 2), ("tr", 1),
                       ("p1", 3), ("tr", 2), ("p1", 4), ("tr", 3)]:
                emit_filler(ev)

            for i in range(NBLK):
                steps = [(c, j) for c in range(4) for j in range(4 * (i + 1))]
                fl = list(fillers[i])
                nst = len(steps)
                # one filler roughly every nst/nfl steps
                stride = max(1, nst // max(1, len(fl)))
                for n, (c, j) in enumerate(steps):
                    st = Step()
                    st.i, st.c, st.j = i, c, j
                    emit_S(st)
                    pv_queue.append(st)
                    if len(pv_queue) > L_PV:
                        drain_one_pv()
                    run_norms()
                    if fl and (n % stride == stride - 1 or n == nst - 1):
                        # pop the first ready filler; at block end flush all
                        while fl:
                            k = next((x for x in range(len(fl))
                                      if filler_ready(fl[x])), None)
                            if k is None:
                                break
                            emit_filler(fl.pop(k))
                            if n != nst - 1:
                                break
                # unready leftovers (WO waiting on this block's own norms):
                # drain pipeline enough, then emit
                while fl:
                    while pv_queue and not all(filler_ready(x) for x in fl):
                        drain_one_pv()
                        run_norms()
                    while norm_queue and not all(filler_ready(x) for x in fl):
                        run_norms()
                    k = next((x for x in range(len(fl))
                              if filler_ready(fl[x])), None)
                    if k is not None:
                        emit_filler(fl.pop(k))
                    else:
                        break
                assert not fl, f"unplaceable fillers in block {i}: {fl}"
            while pv_queue:
                drain_one_pv()
                run_norms()
            while norm_queue:
                run_norms()
            # tail: block 3 out-proj
            for tl in range(4):
                for h in range(2):
                    emit_wo(3, tl, h, block_obufs[3])

    nc.compile()
    return nc


def _host_inputs(x, Wq, Wk, Wv, Wo, q_ln_w, k_ln_w):
    x = np.asarray(x, np.float32)
    Wq = np.asarray(Wq, np.float32)
    Wk = np.asarray(Wk, np.float32)
    Wv = np.asarray(Wv, np.float32)
    Wo = np.asarray(Wo, np.float32)
    q_ln_w = np.asarray(q_ln_w, np.float64)
    k_ln_w = np.asarray(k_ln_w, np.float64)

    inv_freq = 1.0 / (1e6 ** (np.arange(0, D, 2, dtype=np.float64) / D))
    t = np.arange(T, dtype=np.float64)
    freqs = np.outer(t, inv_freq)
    emb = np.concatenate([freqs, freqs], -1)
    cos, sin = np.cos(emb), np.sin(emb)
    rot = (np.arange(D) + 32) % D
    sign = np.where(np.arange(D) < 32, -1.0, 1.0)

    def rope_tab(w):
        cw = w[None, :] * cos
        sw = sign[None, :] * w[rot][None, :] * sin
        return np.concatenate([cw, sw], -1).astype(np.float16)

    def relayout(tab):
        # [T,128] -> [128, 16*128]: row p holds tile tt's row (tt*128+p)
        return np.ascontiguousarray(
            tab.reshape(NTT, P, 128).transpose(1, 0, 2).reshape(P, NTT * 128))

    ropeq = relayout(rope_tab(q_ln_w))
    ropek = relayout(rope_tab(k_ln_w))
    pp_, gg_ = np.meshgrid(np.arange(P), np.arange(P), indexing="ij")
    negI = np.where(pp_ == gg_, MASKVAL, 0.0)
    ustr = (pp_ > gg_).astype(np.float64)
    mtab = np.concatenate([negI, ustr], axis=1).astype(np.float16)

    in_maps = []
    for core in range(8):
        b, g = core // 4, core % 4
        xT = np.ascontiguousarray(x[b].T).astype(np.float16)
        # retile so projection tile tt is one contiguous 512KB row-block:
        # xtt[tt*128+p, hc*128+t'] = xT[hc*128+p, tt*128+t']
        xtt = np.ascontiguousarray(
            xT.reshape(NHC, P, NTT, P).transpose(2, 1, 0, 3).reshape(T, H))
        heads = []
        for c in range(4):
            heads += [g * 8 + c, g * 8 + c + 4]
        wqkv = np.ascontiguousarray(np.concatenate(
            [Wq[:, h * D:(h + 1) * D] for h in heads]
            + [Wk[:, g * 128:(g + 1) * 128], Wv[:, g * 128:(g + 1) * 128]],
            axis=1)).astype(np.float16)
        wo = np.ascontiguousarray(
            np.concatenate([Wo[h * D:(h + 1) * D, :] for h in heads], axis=0)
        ).astype(np.float16)
        in_maps.append({
            "xtt": xtt, "wqkv": wqkv, "wo": wo,
            "ropeq": ropeq, "ropek": ropek, "mtab": mtab,
        })
    return in_maps


def get_program():
    if "nc" not in _CACHE:
        _CACHE["nc"] = _build_program()
    return _CACHE["nc"]


def run(inputs, trace=False, tmpdir=None):
    nc = get_program()
    in_maps = _host_inputs(**inputs)
    res = run_bass_kernel_spmd(nc, in_maps, list(range(8)), trace=trace, tmpdir=tmpdir)
    out = np.zeros((2, T, H), np.float32)
    for core in range(8):
        out[core // 4] += res.results[core]["out"].astype(np.float32)
    return out, res


def kernel(**inputs) -> np.ndarray:
    out, _ = run(inputs, trace=False)
    return out
